# revision 50
# baseline (speedup 1.0000x reference)
"""AxialAttention TRN2 kernel (v2: all-bf16, software-pipelined, fused
output projection; cost-model 626 us vs 763 us for the 900-us baseline).

Sharding: 8 cores = 4 batches x 2 head-groups (4 heads each). Each core:
  phase 1: qkv projection, all-bf16 (x cast on host). No bias matmuls:
           bk cancels in softmax exactly, bv folds into bout host-side
           (post-softmax rows sum to 1), bq (k half zero-padded) is added
           on DVE during the PSUM drain. v drained via ACT copy. qkv written
           to DRAM row-major in 4 quarter tiles (qkA*) and q,k col-major
           (qkB) so attention can stream both axes.
  phase 2 row: per 8-row chunk: transpose-DMA q,k -> scores k^T q -> exp
           (no max-subtraction, scores bounded) -> hl-stacked Z via
           ones-matmul -> one reciprocal per head-pair -> PV -> drain
           psO * rz contiguously into O_sb.
  phase 2 col + fused output projection: mirror of row reading qkB, whose
           v block loads with a single reordered-AP DMA per 8-column chunk. Drain:
           tmp = psO * rz on DVE, then gpsimd adds tmp into O_sb at the
           strided (col-major) positions. As soon as a column chunk is
           complete in O_sb, its output projection runs (strided matmul rhs)
           and the result is written bf16 col-major; the host un-permutes.
Host: per-batch sum of the two head-group partial outputs, then (C,W,H) ->
  (C,H,W) transpose.

DMA routing: nc.sync carries ONLY transpose DMAs (XBAR transpose<->copy
interleave corrupts data on this HW), nc.scalar the phase-1 streams and
output, nc.gpsimd (SWDGE) the v streams.
"""

import numpy as np
import ml_dtypes
from contextlib import ExitStack

import concourse.bass as bass
import concourse.bacc as bacc
import concourse.tile as tile
from concourse import mybir
from concourse.bass_utils import run_bass_kernel_spmd

C = 512          # channels
H = 128          # height
W = 128          # width
S = H * W        # 16384 pixels
NH = 8           # total heads
D = 64           # head dim
NHC = 4          # heads per core
GC = NHC * D     # 256 group channels (q or k or v)
QKV = 3 * GC     # 768 projected channels per core
CT = C // 128    # 4 contraction tiles
ST = S // 128    # 128 spatial tiles
NQ = 4           # qkA quarter tiles
SCALE = 1.0 / np.sqrt(D)

F32 = mybir.dt.float32
BF16 = mybir.dt.bfloat16
EXP = mybir.ActivationFunctionType.Exp
IDENT = mybir.ActivationFunctionType.Identity
ADD = mybir.AluOpType.add
MULT = mybir.AluOpType.mult

_CACHED_NC = None


def build_nc(reps=1):
    nc = bacc.Bacc()
    x_in = nc.dram_tensor("x", [C, S], BF16, kind="ExternalInput")
    wqkvT = nc.dram_tensor("wqkvT", [C, QKV], BF16, kind="ExternalInput")
    bqk = nc.dram_tensor("bqk", [128, 512], BF16, kind="ExternalInput")
    woutT = nc.dram_tensor("woutT", [GC, C], BF16, kind="ExternalInput")
    bout = nc.dram_tensor("bout", [128, CT], F32, kind="ExternalInput")
    out = nc.dram_tensor("out", [C, S], BF16, kind="ExternalOutput")

    with tile.TileContext(nc) as tc, ExitStack() as ctx:
        persist = ctx.enter_context(tc.tile_pool(name="persist", bufs=1))
        dram = ctx.enter_context(tc.tile_pool(name="dram", bufs=1, space="DRAM"))

        w_sb = persist.tile([128, CT, QKV], BF16, tag="w_sb")
        nc.scalar.dma_start(
            out=w_sb, in_=wqkvT.ap().rearrange("(t p) o -> p t o", p=128)
        )
        bqk_sb = persist.tile([128, 512], BF16, tag="bqk_sb")
        nc.scalar.dma_start(out=bqk_sb, in_=bqk.ap())
        wout_sb = persist.tile([128, 2, C], BF16, tag="wout_sb")
        nc.scalar.dma_start(
            out=wout_sb, in_=woutT.ap().rearrange("(t p) o -> p t o", p=128)
        )
        boutv = persist.tile([128, CT], F32, tag="boutv")
        nc.scalar.dma_start(out=boutv, in_=bout.ap())
        ones_sb = persist.tile([128, 128], BF16, tag="ones_sb")
        nc.vector.memset(ones_sb, 1.0)

        O_sb = [
            persist.tile([128, S], BF16, tag=f"O{i}", name=f"O{i}") for i in range(2)
        ]

        # row-major qkv in quarters (pixel order s = h*W + w)
        qkvQ = [
            dram.tile([S // NQ, QKV], BF16, tag=f"qkvQ{i}", name=f"qkvQ{i}")
            for i in range(NQ)
        ]
        qkB = dram.tile([S, QKV], BF16)   # q,k,v; pixel order s' = w*H + h

        for _rep in range(reps):
            build_body(nc, tc, x_in, w_sb, bqk_sb, wout_sb, boutv,
                       ones_sb, O_sb, qkvQ, qkB, out)

    nc.finalize()
    return nc


def build_body(nc, tc, x_in, w_sb, bqk_sb, wout_sb, boutv, ones_sb,
               O_sb, qkvQ, qkB, out):
    # ---------- phase 1: qkv projection (x stationary, s-major out) ----------
    x_r = x_in.ap().rearrange("(t p) s -> p t s", p=128)
    with (
        tc.tile_pool(name="p1x", bufs=3) as xpool,
        tc.tile_pool(name="p1ps", bufs=3, space="PSUM") as pspool,
        tc.tile_pool(name="p1o", bufs=3) as opool,
    ):
        for sg in range(ST // 4):  # groups of 4 s-tiles (one h-quad)
            if sg % 2 == 0:
                xg = xpool.tile([128, CT, 1024], BF16)
                nc.scalar.dma_start(
                    out=xg, in_=x_r[:, :, sg * 512 : sg * 512 + 1024]
                )
            qt4 = opool.tile([128, 4, QKV], BF16)
            for i in range(4):
                ps = pspool.tile([128, QKV], F32)
                for ct in range(CT):
                    lhsT = xg[
                        :, ct,
                        (sg % 2) * 512 + i * 128 : (sg % 2) * 512 + (i + 1) * 128,
                    ]
                    nc.tensor.matmul(
                        out=ps[:, 0:512], lhsT=lhsT, rhs=w_sb[:, ct, 0:512],
                        start=(ct == 0), stop=(ct == CT - 1),
                    )
                    nc.tensor.matmul(
                        out=ps[:, 512:QKV], lhsT=lhsT, rhs=w_sb[:, ct, 512:QKV],
                        start=(ct == 0), stop=(ct == CT - 1),
                    )
                # q,k: add bias (k half zero) on DVE; v: plain copy on ACT
                nc.vector.tensor_tensor(
                    out=qt4[:, i, 0:512], in0=ps[:, 0:512], in1=bqk_sb, op=ADD
                )
                nc.scalar.copy(out=qt4[:, i, 512:QKV], in_=ps[:, 512:QKV])
            # rows (sg*4+i)*128 + w of the row-major copy -> quarter sg//8
            qi, sgq = sg // 8, sg % 8
            dstA = qkvQ[qi][sgq * 512 : (sgq + 1) * 512, :].rearrange(
                "(i p) o -> p i o", p=128
            )
            nc.scalar.dma_start(out=dstA, in_=qt4)
            # rows w*128 + (sg*4+i) of the col-major q,k copy
            dstB = qkB[:].rearrange("(p i) o -> p i o", i=ST)[
                :, sg * 4 : (sg + 1) * 4, :
            ]
            nc.scalar.dma_start(out=dstB, in_=qt4)

    # ---------- phase 2: axial attention, software-pipelined ----------
    # branch 0 = row (writes O_sb contiguously), branch 1 = col (adds into
    # O_sb at strided col-major positions via gpsimd, then runs the fused
    # output projection on each finished 8-column chunk, one tg delayed).
    # Chunk c = (tg, hp, j): 4 attention tiles. Emission is skewed: loads
    # lead by 2 chunks, scores/exp by 1, so the in-order PE queue never
    # waits on exp (it runs during the next chunk's score matmuls).
    out_r = out.ap().rearrange("(t p) s -> p t s", p=128)
    NCH = (ST // 8) * 4  # chunks per branch

    def proj_tg(tg, ofpool, psumF):
        """Output projection compute for the 8 finished columns of col-tg."""
        of4s = []
        for ch in range(2):  # two 512-px' chunks
            w0 = tg * 8 + ch * 4
            of4 = ofpool.tile([128, CT, 512], BF16)
            for ot in range(CT):
                psF = psumF.tile([128, 512], F32)
                for hp in range(2):
                    rhs = O_sb[hp][:, :].rearrange(
                        "p (h w) -> p w h", w=W
                    )[:, w0 : w0 + 4, :]
                    nc.tensor.matmul(
                        out=psF,
                        lhsT=wout_sb[:, hp, ot * 128 : (ot + 1) * 128],
                        rhs=rhs,
                        start=(hp == 0), stop=(hp == 1),
                    )
                if ot < 2:
                    nc.scalar.activation(
                        out=of4[:, ot, :], in_=psF, func=IDENT,
                        bias=boutv[:, ot : ot + 1], scale=1.0,
                    )
                else:
                    nc.vector.tensor_scalar_add(
                        out=of4[:, ot, :], in0=psF,
                        scalar1=boutv[:, ot : ot + 1],
                    )
            nc.scalar.dma_start(
                out=out_r[:, :, w0 * 128 : (w0 + 4) * 128], in_=of4
            )

    for branch in range(2):
      pfx = "ab"[branch]
      with (
        tc.tile_pool(name=f"{pfx}_qt", bufs=4) as qtpool,
        tc.tile_pool(name=f"{pfx}_kt", bufs=4) as ktpool,
        tc.tile_pool(name=f"{pfx}_vt", bufs=2) as vtpool,
        tc.tile_pool(name=f"{pfx}_p", bufs=3) as ppool,
        tc.tile_pool(name=f"{pfx}_rz", bufs=2) as rzpool,
        tc.tile_pool(name=f"{pfx}_tm", bufs=3) as tmpool,
        tc.tile_pool(name=f"{pfx}_of", bufs=2) as ofpool,
        tc.tile_pool(name=f"{pfx}_psS", bufs=2, space="PSUM") as psumS,
        tc.tile_pool(name=f"{pfx}_psZ", bufs=1, space="PSUM") as psumZ,
        tc.tile_pool(
            name=f"{pfx}_psO", bufs=(2 if branch == 0 else 1), space="PSUM"
        ) as psumO,
        tc.tile_pool(name=f"{pfx}_psF", bufs=2, space="PSUM") as psumF,
      ):
        qk8 = {}   # (tg, hp) -> (q8, k8)
        vt = {}    # tg -> vt8
        st = {}    # chunk -> (pch0, pch1)

        def rows_of(tg):
            if branch == 0:
                return qkvQ[tg // 4][(tg % 4) * 1024 : (tg % 4 + 1) * 1024, :]
            return qkB[tg * 1024 : (tg + 1) * 1024, :]

        def load_qk(c):
            tg, hp = c // 4, (c // 2) % 2
            rows = rows_of(tg)
            if (tg, hp) not in qk8:
                q8 = qtpool.tile([128, 1024], BF16)
                nc.sync.dma_start_transpose(
                    out=q8, in_=rows[:, hp * 128 : (hp + 1) * 128]
                )
                k8 = ktpool.tile([128, 1024], BF16)
                nc.sync.dma_start_transpose(
                    out=k8, in_=rows[:, 256 + hp * 128 : 256 + (hp + 1) * 128]
                )
                qk8[(tg, hp)] = (q8, k8)

        def load_vt(c):
            tg = c // 4
            rows = rows_of(tg)
            if tg not in vt:
                vt8 = vtpool.tile([128, 8, 256], BF16)
                if branch == 0:
                    nc.gpsimd.dma_start(
                        out=vt8,
                        in_=rows[:, 512:768].rearrange("(i p) o -> p i o", p=128),
                    )
                else:
                    # v for col tiles w = tg*8+i sits in qkB rows
                    # [tg*1024, (tg+1)*1024) cols 512:768; reorder the DRAM
                    # AP (w h) c -> h w c so one DMA fills [h, w, ch]
                    src = rows[:, 512:768].rearrange("(w h) o -> h w o", h=H)
                    nc.gpsimd.dma_start(out=vt8, in_=src)
                vt[tg] = vt8

        for c in range(NCH + 1):
            if c < NCH:
                for la in range(6):
                    if c + la < NCH:
                        load_qk(c + la)
                        if la < 3:
                            load_vt(c + la)
                # scores + ONE exp for chunk c (both heads side by side)
                tg, hp, j = c // 4, (c // 2) % 2, c % 2
                q8, k8 = qk8[(tg, hp)]
                psS = psumS.tile([128, 1024], F32)
                for hl in range(2):
                    r0, r1 = hl * 64, (hl + 1) * 64
                    for i in range(4):
                        ii = j * 4 + i
                        nc.tensor.matmul(
                            out=psS[:, hl * 512 + i * 128 : hl * 512 + (i + 1) * 128],
                            lhsT=k8[r0:r1, ii * 128 : (ii + 1) * 128],
                            rhs=q8[r0:r1, ii * 128 : (ii + 1) * 128],
                            start=True, stop=True,
                        )
                pch = ppool.tile([128, 1024], BF16)
                nc.scalar.activation(
                    out=pch, in_=psS, func=EXP, scale=float(SCALE)
                )
                st[c] = pch
            if c >= 1:
                # Z, PV, normalize, drain for chunk c-1
                d = c - 1
                tg, hp, j = d // 4, (d // 2) % 2, d % 2
                pch = st.pop(d)
                vt8 = vt[tg]
                psO = psumO.tile([128, 512], F32)
                psZ = psumZ.tile([128, 512], F32)
                for hl in range(2):
                    r0, r1 = hl * 64, (hl + 1) * 64
                    nc.tensor.matmul(
                        out=psZ[r0:r1, :], lhsT=ones_sb[:, 0:64],
                        rhs=pch[:, hl * 512 : (hl + 1) * 512],
                        start=True, stop=True,
                    )
                    for i in range(4):
                        ii = j * 4 + i
                        nc.tensor.matmul(
                            out=psO[r0:r1, i * 128 : (i + 1) * 128],
                            lhsT=vt8[:, ii, hp * 128 + r0 : hp * 128 + r1],
                            rhs=pch[:, hl * 512 + i * 128 : hl * 512 + (i + 1) * 128],
                            start=True, stop=True,
                        )
                rz = rzpool.tile([128, 512], F32)
                nc.vector.reciprocal_approx_fast(out=rz, in_=psZ)
                t0 = tg * 8 + j * 4
                if branch == 0:
                    nc.vector.tensor_tensor(
                        out=O_sb[hp][:, t0 * 128 : t0 * 128 + 512],
                        in0=psO, in1=rz, op=MULT,
                    )
                else:
                    tm = tmpool.tile([128, 512], BF16)
                    nc.vector.tensor_tensor(
                        out=tm, in0=psO, in1=rz, op=MULT,
                    )
                    dst = O_sb[hp][:, :].rearrange(
                        "p (h w) -> p w h", w=W
                    )[:, t0 : t0 + 4, :]
                    nc.gpsimd.tensor_tensor(
                        out=dst, in0=dst,
                        in1=tm.rearrange("p (w h) -> p w h", w=4), op=ADD,
                    )
                # fused projection, one tg delayed so the gpsimd adds of
                # that tg are long done (no PE stall)
                if branch == 1 and d % 4 == 3 and tg >= 1:
                    proj_tg(tg - 1, ofpool, psumF)
        if branch == 1:
            proj_tg(ST // 8 - 1, ofpool, psumF)


def get_nc():
    global _CACHED_NC
    if _CACHED_NC is None:
        _CACHED_NC = build_nc()
    return _CACHED_NC


def make_in_maps(x, Wqkv, bqkv, Wout, bout):
    """Per-core input dicts: core c = (b, g) with b = c // 2, g = c % 2."""
    bf16 = ml_dtypes.bfloat16
    in_maps = []
    for c in range(8):
        b, g = c // 2, c % 2
        sel = slice(256 * g, 256 * (g + 1))
        wsel = np.concatenate(
            [Wqkv[sel, :], Wqkv[512 + 256 * g : 512 + 256 * (g + 1), :],
             Wqkv[1024 + 256 * g : 1024 + 256 * (g + 1), :]], axis=0
        )  # [768, 512]
        bq = bqkv[sel]                                   # [256] q bias
        bv = bqkv[1024 + 256 * g : 1024 + 256 * (g + 1)]  # [256] v bias
        bqk = np.zeros((128, 512), np.float32)
        bqk[:, 0:256] = bq[None, :]
        woutT = np.ascontiguousarray(Wout[:, sel].T)  # [256, 512]
        # bout correction: post-softmax P rows sum to 1, so dropping bv from
        # phase 1 removes exactly 2*Wout_g@bv_g (row+col) from the output.
        bcorr = 2.0 * (Wout[:, sel].astype(np.float64) @ bv.astype(np.float64))
        bfull = bcorr + (bout.astype(np.float64) if g == 0 else 0.0)
        in_maps.append(
            {
                "x": np.ascontiguousarray(x[b].reshape(C, S)).astype(bf16),
                "wqkvT": np.ascontiguousarray(wsel.T).astype(bf16),
                "bqk": bqk.astype(bf16),
                "woutT": woutT.astype(bf16),
                "bout": np.ascontiguousarray(
                    bfull.astype(np.float32).reshape(CT, 128).T
                ),
            }
        )
    return in_maps


def kernel(x, Wqkv, bqkv, Wout, bout):
    x = np.asarray(x, dtype=np.float32)
    Wqkv = np.asarray(Wqkv, dtype=np.float32)
    bqkv = np.asarray(bqkv, dtype=np.float32)
    Wout = np.asarray(Wout, dtype=np.float32)
    bout = np.asarray(bout, dtype=np.float32)

    nc = get_nc()
    in_maps = make_in_maps(x, Wqkv, bqkv, Wout, bout)
    res = run_bass_kernel_spmd(nc, in_maps, core_ids=list(range(8)))
    B = x.shape[0]
    out = np.empty((B, C, H, W), dtype=np.float32)
    for b in range(B):
        acc = (res.results[2 * b]["out"].astype(np.float32)
               + res.results[2 * b + 1]["out"].astype(np.float32))
        # device wrote col-major (s' = w*H + h): un-permute
        out[b] = acc.reshape(C, W, H).transpose(0, 2, 1)
    return out


# revision 51
# speedup vs baseline: 1.0448x; 1.0448x over previous
"""AxialAttention TRN2 kernel (v2: all-bf16, software-pipelined, fused
output projection; cost-model 605 us vs 763 us for the 900-us baseline).

Sharding: 8 cores = 4 batches x 2 head-groups (4 heads each). Each core:
  phase 1: qkv projection, all-bf16 (x cast on host). No bias matmuls:
           bk cancels in softmax exactly, bv folds into bout host-side
           (post-softmax rows sum to 1), bq (k half zero-padded) is added
           on DVE during the PSUM drain. v drained via ACT copy. qkv written
           to DRAM row-major in 4 quarter tiles (qkA*) and q,k col-major
           (qkB) so attention can stream both axes.
  phase 2 row: per 4-tile chunk (software-pipelined, scores/exp lead
           Z/PV by one chunk): transpose-DMA q,k -> scores k^T q, both heads
           packed into one 2-bank PSUM tile -> ONE exp per chunk (no
           max-subtraction, scores bounded) -> hl-stacked Z via ones-matmul
           -> one reciprocal per chunk -> PV -> drain psO * rz into O_sb.
  phase 2 col + fused output projection: mirror of row reading qkB, whose
           v block loads with a single reordered-AP DMA per 8-column chunk. Drain:
           tmp = psO * rz on DVE, then gpsimd adds tmp into O_sb at the
           strided (col-major) positions. As soon as a column chunk is
           complete in O_sb, its output projection runs (strided matmul rhs)
           and the result is written bf16 col-major; the host un-permutes.
Host: per-batch sum of the two head-group partial outputs, then (C,W,H) ->
  (C,H,W) transpose.

DMA routing: nc.sync carries ONLY transpose DMAs (XBAR transpose<->copy
interleave corrupts data on this HW), nc.scalar the phase-1 streams and
output, nc.gpsimd (SWDGE) the v streams.
"""

import numpy as np
import ml_dtypes
from contextlib import ExitStack

import concourse.bass as bass
import concourse.bacc as bacc
import concourse.tile as tile
from concourse import mybir
from concourse.bass_utils import run_bass_kernel_spmd

C = 512          # channels
H = 128          # height
W = 128          # width
S = H * W        # 16384 pixels
NH = 8           # total heads
D = 64           # head dim
NHC = 4          # heads per core
GC = NHC * D     # 256 group channels (q or k or v)
QKV = 3 * GC     # 768 projected channels per core
CT = C // 128    # 4 contraction tiles
ST = S // 128    # 128 spatial tiles
NQ = 4           # qkA quarter tiles
SCALE = 1.0 / np.sqrt(D)

F32 = mybir.dt.float32
BF16 = mybir.dt.bfloat16
EXP = mybir.ActivationFunctionType.Exp
IDENT = mybir.ActivationFunctionType.Identity
ADD = mybir.AluOpType.add
MULT = mybir.AluOpType.mult

_CACHED_NC = None


def build_nc(reps=1):
    nc = bacc.Bacc()
    x_in = nc.dram_tensor("x", [C, S], BF16, kind="ExternalInput")
    wqkvT = nc.dram_tensor("wqkvT", [C, QKV], BF16, kind="ExternalInput")
    bqk = nc.dram_tensor("bqk", [128, 512], BF16, kind="ExternalInput")
    woutT = nc.dram_tensor("woutT", [GC, C], BF16, kind="ExternalInput")
    bout = nc.dram_tensor("bout", [128, CT], F32, kind="ExternalInput")
    out = nc.dram_tensor("out", [C, S], BF16, kind="ExternalOutput")

    with tile.TileContext(nc) as tc, ExitStack() as ctx:
        persist = ctx.enter_context(tc.tile_pool(name="persist", bufs=1))
        dram = ctx.enter_context(tc.tile_pool(name="dram", bufs=1, space="DRAM"))

        w_sb = persist.tile([128, CT, QKV], BF16, tag="w_sb")
        nc.scalar.dma_start(
            out=w_sb, in_=wqkvT.ap().rearrange("(t p) o -> p t o", p=128)
        )
        bqk_sb = persist.tile([128, 512], BF16, tag="bqk_sb")
        nc.scalar.dma_start(out=bqk_sb, in_=bqk.ap())
        wout_sb = persist.tile([128, 2, C], BF16, tag="wout_sb")
        nc.scalar.dma_start(
            out=wout_sb, in_=woutT.ap().rearrange("(t p) o -> p t o", p=128)
        )
        boutv = persist.tile([128, CT], F32, tag="boutv")
        nc.scalar.dma_start(out=boutv, in_=bout.ap())
        ones_sb = persist.tile([128, 128], BF16, tag="ones_sb")
        nc.vector.memset(ones_sb, 1.0)

        O_sb = [
            persist.tile([128, S], BF16, tag=f"O{i}", name=f"O{i}") for i in range(2)
        ]

        # row-major qkv in quarters (pixel order s = h*W + w)
        qkvQ = [
            dram.tile([S // NQ, QKV], BF16, tag=f"qkvQ{i}", name=f"qkvQ{i}")
            for i in range(NQ)
        ]
        qkB = dram.tile([S, QKV], BF16)   # q,k,v; pixel order s' = w*H + h

        for _rep in range(reps):
            build_body(nc, tc, x_in, w_sb, bqk_sb, wout_sb, boutv,
                       ones_sb, O_sb, qkvQ, qkB, out)

    nc.finalize()
    return nc


def build_body(nc, tc, x_in, w_sb, bqk_sb, wout_sb, boutv, ones_sb,
               O_sb, qkvQ, qkB, out):
    # ---------- phase 1: qkv projection (x stationary, s-major out) ----------
    x_r = x_in.ap().rearrange("(t p) s -> p t s", p=128)
    with (
        tc.tile_pool(name="p1x", bufs=3) as xpool,
        tc.tile_pool(name="p1ps", bufs=3, space="PSUM") as pspool,
        tc.tile_pool(name="p1o", bufs=3) as opool,
    ):
        for sg in range(ST // 4):  # groups of 4 s-tiles (one h-quad)
            if sg % 2 == 0:
                xg = xpool.tile([128, CT, 1024], BF16)
                nc.scalar.dma_start(
                    out=xg, in_=x_r[:, :, sg * 512 : sg * 512 + 1024]
                )
            qt4 = opool.tile([128, 4, QKV], BF16)
            for i in range(4):
                ps = pspool.tile([128, QKV], F32)
                for ct in range(CT):
                    lhsT = xg[
                        :, ct,
                        (sg % 2) * 512 + i * 128 : (sg % 2) * 512 + (i + 1) * 128,
                    ]
                    nc.tensor.matmul(
                        out=ps[:, 0:512], lhsT=lhsT, rhs=w_sb[:, ct, 0:512],
                        start=(ct == 0), stop=(ct == CT - 1),
                    )
                    nc.tensor.matmul(
                        out=ps[:, 512:QKV], lhsT=lhsT, rhs=w_sb[:, ct, 512:QKV],
                        start=(ct == 0), stop=(ct == CT - 1),
                    )
                # q,k: add bias (k half zero) on DVE; v: plain copy on ACT
                nc.vector.tensor_tensor(
                    out=qt4[:, i, 0:512], in0=ps[:, 0:512], in1=bqk_sb, op=ADD
                )
                nc.scalar.copy(out=qt4[:, i, 512:QKV], in_=ps[:, 512:QKV])
            # rows (sg*4+i)*128 + w of the row-major copy -> quarter sg//8
            qi, sgq = sg // 8, sg % 8
            dstA = qkvQ[qi][sgq * 512 : (sgq + 1) * 512, :].rearrange(
                "(i p) o -> p i o", p=128
            )
            nc.scalar.dma_start(out=dstA, in_=qt4)
            # rows w*128 + (sg*4+i) of the col-major q,k copy
            dstB = qkB[:].rearrange("(p i) o -> p i o", i=ST)[
                :, sg * 4 : (sg + 1) * 4, :
            ]
            nc.scalar.dma_start(out=dstB, in_=qt4)

    # ---------- phase 2: axial attention, software-pipelined ----------
    # branch 0 = row (writes O_sb contiguously), branch 1 = col (adds into
    # O_sb at strided col-major positions via gpsimd, then runs the fused
    # output projection on each finished 8-column chunk, one tg delayed).
    # Chunk c = (tg, hp, j): 4 attention tiles. Emission is skewed: loads
    # lead by 2 chunks, scores/exp by 1, so the in-order PE queue never
    # waits on exp (it runs during the next chunk's score matmuls).
    out_r = out.ap().rearrange("(t p) s -> p t s", p=128)
    NCH = (ST // 8) * 4  # chunks per branch

    def proj_tg(tg, ofpool, psumF):
        """Output projection compute for the 8 finished columns of col-tg."""
        of4s = []
        for ch in range(2):  # two 512-px' chunks
            w0 = tg * 8 + ch * 4
            of4 = ofpool.tile([128, CT, 512], BF16)
            for ot in range(CT):
                psF = psumF.tile([128, 512], F32)
                for hp in range(2):
                    rhs = O_sb[hp][:, :].rearrange(
                        "p (h w) -> p w h", w=W
                    )[:, w0 : w0 + 4, :]
                    nc.tensor.matmul(
                        out=psF,
                        lhsT=wout_sb[:, hp, ot * 128 : (ot + 1) * 128],
                        rhs=rhs,
                        start=(hp == 0), stop=(hp == 1),
                    )
                if ot < 2:
                    nc.scalar.activation(
                        out=of4[:, ot, :], in_=psF, func=IDENT,
                        bias=boutv[:, ot : ot + 1], scale=1.0,
                    )
                else:
                    nc.vector.tensor_scalar_add(
                        out=of4[:, ot, :], in0=psF,
                        scalar1=boutv[:, ot : ot + 1],
                    )
            nc.scalar.dma_start(
                out=out_r[:, :, w0 * 128 : (w0 + 4) * 128], in_=of4
            )

    for branch in range(2):
      pfx = "ab"[branch]
      with (
        tc.tile_pool(name=f"{pfx}_qt", bufs=4) as qtpool,
        tc.tile_pool(name=f"{pfx}_kt", bufs=4) as ktpool,
        tc.tile_pool(name=f"{pfx}_vt", bufs=2) as vtpool,
        tc.tile_pool(name=f"{pfx}_p", bufs=3) as ppool,
        tc.tile_pool(name=f"{pfx}_rz", bufs=2) as rzpool,
        tc.tile_pool(name=f"{pfx}_tm", bufs=3) as tmpool,
        tc.tile_pool(name=f"{pfx}_of", bufs=2) as ofpool,
        tc.tile_pool(name=f"{pfx}_psS", bufs=2, space="PSUM") as psumS,
        tc.tile_pool(name=f"{pfx}_psZ", bufs=1, space="PSUM") as psumZ,
        tc.tile_pool(
            name=f"{pfx}_psO", bufs=(2 if branch == 0 else 1), space="PSUM"
        ) as psumO,
        tc.tile_pool(name=f"{pfx}_psF", bufs=2, space="PSUM") as psumF,
      ):
        qk8 = {}   # (tg, hp) -> (q8, k8)
        vt = {}    # tg -> vt8
        st = {}    # chunk -> (pch0, pch1)

        def rows_of(tg):
            if branch == 0:
                return qkvQ[tg // 4][(tg % 4) * 1024 : (tg % 4 + 1) * 1024, :]
            return qkB[tg * 1024 : (tg + 1) * 1024, :]

        def load_qk(c):
            tg, hp = c // 4, (c // 2) % 2
            rows = rows_of(tg)
            if (tg, hp) not in qk8:
                q8 = qtpool.tile([128, 1024], BF16)
                nc.sync.dma_start_transpose(
                    out=q8, in_=rows[:, hp * 128 : (hp + 1) * 128]
                )
                k8 = ktpool.tile([128, 1024], BF16)
                nc.sync.dma_start_transpose(
                    out=k8, in_=rows[:, 256 + hp * 128 : 256 + (hp + 1) * 128]
                )
                qk8[(tg, hp)] = (q8, k8)

        def load_vt(c):
            tg = c // 4
            rows = rows_of(tg)
            if tg not in vt:
                vt8 = vtpool.tile([128, 8, 256], BF16)
                if branch == 0:
                    nc.gpsimd.dma_start(
                        out=vt8,
                        in_=rows[:, 512:768].rearrange("(i p) o -> p i o", p=128),
                    )
                else:
                    # v for col tiles w = tg*8+i sits in qkB rows
                    # [tg*1024, (tg+1)*1024) cols 512:768; reorder the DRAM
                    # AP (w h) c -> h w c so one DMA fills [h, w, ch]
                    src = rows[:, 512:768].rearrange("(w h) o -> h w o", h=H)
                    nc.gpsimd.dma_start(out=vt8, in_=src)
                vt[tg] = vt8

        for c in range(NCH + 1):
            if c < NCH:
                for la in range(6):
                    if c + la < NCH:
                        load_qk(c + la)
                        if la < 3:
                            load_vt(c + la)
                # scores + ONE exp for chunk c (both heads side by side)
                tg, hp, j = c // 4, (c // 2) % 2, c % 2
                q8, k8 = qk8[(tg, hp)]
                psS = psumS.tile([128, 1024], F32)
                for hl in range(2):
                    r0, r1 = hl * 64, (hl + 1) * 64
                    for i in range(4):
                        ii = j * 4 + i
                        nc.tensor.matmul(
                            out=psS[:, hl * 512 + i * 128 : hl * 512 + (i + 1) * 128],
                            lhsT=k8[r0:r1, ii * 128 : (ii + 1) * 128],
                            rhs=q8[r0:r1, ii * 128 : (ii + 1) * 128],
                            start=True, stop=True,
                        )
                pch = ppool.tile([128, 1024], BF16)
                nc.scalar.activation(
                    out=pch, in_=psS, func=EXP, scale=float(SCALE)
                )
                st[c] = pch
            if c >= 1:
                # Z, PV, normalize, drain for chunk c-1
                d = c - 1
                tg, hp, j = d // 4, (d // 2) % 2, d % 2
                pch = st.pop(d)
                vt8 = vt[tg]
                psO = psumO.tile([128, 512], F32)
                psZ = psumZ.tile([128, 512], F32)
                for hl in range(2):
                    r0, r1 = hl * 64, (hl + 1) * 64
                    nc.tensor.matmul(
                        out=psZ[r0:r1, :], lhsT=ones_sb[:, 0:64],
                        rhs=pch[:, hl * 512 : (hl + 1) * 512],
                        start=True, stop=True,
                    )
                    for i in range(4):
                        ii = j * 4 + i
                        nc.tensor.matmul(
                            out=psO[r0:r1, i * 128 : (i + 1) * 128],
                            lhsT=vt8[:, ii, hp * 128 + r0 : hp * 128 + r1],
                            rhs=pch[:, hl * 512 + i * 128 : hl * 512 + (i + 1) * 128],
                            start=True, stop=True,
                        )
                rz = rzpool.tile([128, 512], F32)
                nc.vector.reciprocal_approx_fast(out=rz, in_=psZ)
                t0 = tg * 8 + j * 4
                if branch == 0:
                    nc.vector.tensor_tensor(
                        out=O_sb[hp][:, t0 * 128 : t0 * 128 + 512],
                        in0=psO, in1=rz, op=MULT,
                    )
                else:
                    tm = tmpool.tile([128, 512], BF16)
                    nc.vector.tensor_tensor(
                        out=tm, in0=psO, in1=rz, op=MULT,
                    )
                    dst = O_sb[hp][:, :].rearrange(
                        "p (h w) -> p w h", w=W
                    )[:, t0 : t0 + 4, :]
                    nc.gpsimd.tensor_tensor(
                        out=dst, in0=dst,
                        in1=tm.rearrange("p (w h) -> p w h", w=4), op=ADD,
                    )
                # fused projection, one tg delayed so the gpsimd adds of
                # that tg are long done (no PE stall)
                if branch == 1 and d % 4 == 3 and tg >= 1:
                    proj_tg(tg - 1, ofpool, psumF)
        if branch == 1:
            proj_tg(ST // 8 - 1, ofpool, psumF)


def get_nc():
    global _CACHED_NC
    if _CACHED_NC is None:
        _CACHED_NC = build_nc()
    return _CACHED_NC


def make_in_maps(x, Wqkv, bqkv, Wout, bout):
    """Per-core input dicts: core c = (b, g) with b = c // 2, g = c % 2."""
    bf16 = ml_dtypes.bfloat16
    in_maps = []
    for c in range(8):
        b, g = c // 2, c % 2
        sel = slice(256 * g, 256 * (g + 1))
        wsel = np.concatenate(
            [Wqkv[sel, :], Wqkv[512 + 256 * g : 512 + 256 * (g + 1), :],
             Wqkv[1024 + 256 * g : 1024 + 256 * (g + 1), :]], axis=0
        )  # [768, 512]
        bq = bqkv[sel]                                   # [256] q bias
        bv = bqkv[1024 + 256 * g : 1024 + 256 * (g + 1)]  # [256] v bias
        bqk = np.zeros((128, 512), np.float32)
        bqk[:, 0:256] = bq[None, :]
        woutT = np.ascontiguousarray(Wout[:, sel].T)  # [256, 512]
        # bout correction: post-softmax P rows sum to 1, so dropping bv from
        # phase 1 removes exactly 2*Wout_g@bv_g (row+col) from the output.
        bcorr = 2.0 * (Wout[:, sel].astype(np.float64) @ bv.astype(np.float64))
        bfull = bcorr + (bout.astype(np.float64) if g == 0 else 0.0)
        in_maps.append(
            {
                "x": np.ascontiguousarray(x[b].reshape(C, S)).astype(bf16),
                "wqkvT": np.ascontiguousarray(wsel.T).astype(bf16),
                "bqk": bqk.astype(bf16),
                "woutT": woutT.astype(bf16),
                "bout": np.ascontiguousarray(
                    bfull.astype(np.float32).reshape(CT, 128).T
                ),
            }
        )
    return in_maps


def kernel(x, Wqkv, bqkv, Wout, bout):
    x = np.asarray(x, dtype=np.float32)
    Wqkv = np.asarray(Wqkv, dtype=np.float32)
    bqkv = np.asarray(bqkv, dtype=np.float32)
    Wout = np.asarray(Wout, dtype=np.float32)
    bout = np.asarray(bout, dtype=np.float32)

    nc = get_nc()
    in_maps = make_in_maps(x, Wqkv, bqkv, Wout, bout)
    res = run_bass_kernel_spmd(nc, in_maps, core_ids=list(range(8)))
    B = x.shape[0]
    out = np.empty((B, C, H, W), dtype=np.float32)
    for b in range(B):
        acc = (res.results[2 * b]["out"].astype(np.float32)
               + res.results[2 * b + 1]["out"].astype(np.float32))
        # device wrote col-major (s' = w*H + h): un-permute
        out[b] = acc.reshape(C, W, H).transpose(0, 2, 1)
    return out
